# revision 7
# baseline (speedup 1.0000x reference)
"""Trainium2 Bass kernel for nn_BidirectionalMLP (8-core SPMD).

Math (from the reference, EPS=0.5, BETA=0.5):
  states stay in [0,1] after every clipped update, so rho(s)=s for all
  state tensors; rx = clip(x,0,1) is fixed.
  Per relaxation step:
    s1' = clip(0.5*s1 + 0.25*(rx@fw0) + 0.25*(s2@bw1), 0, 1)
    s2' = clip(0.5*s2 + 0.25*(s1@fw1 + s3@bw2), 0, 1)
    s3' = clip(0.5*s3 + 0.5*(s2@fw2), 0, 1)              (free phase)
    s3' = clip(0.5*(s2@fw2) + 0.5*y, 0, 1)               (weak phase)
  20 free steps + 5 weak steps from zero states. Step 1 is degenerate:
  s1(1) = clip(0.25*(rx@fw0)), s2(1) = 0, s3(1) = 0 — computed in the
  preamble, so the main loop runs 24 iterations.

Sharding: output-feature sharding of the big weights. Core c owns columns
[512c, 512c+512) of fw1/bw1/bw2, keeping fw1+bw1 SBUF-resident as bf16
(8MB). Each step all-gathers the bf16 states s1 and s2 (two AllGathers
of [512,256] per rank). Phase order alternates each iteration so each
AllGather has a full matmul phase to hide behind:
  phase A: psP1 = s2g@bw1 (+ ps3 = s2g@fw2) -> s1,s3 update -> AG(s1)
  phase B: psP2 = s1g@fw1 + s3@bw2          -> s2 update    -> AG(s2)
Matmuls are weight-stationary: out[feat,batch] tiles [128,256] fp32 in
PSUM, lhsT = weight chunk [128,128] bf16, rhs = gathered state chunk
[128,256] bf16. Loops run m-outer/j-inner so each 128-feature group
finishes early; its 2-op DVE epilogue writes the bf16 state in place and
immediately DMA-stages that chunk for the AllGather, keeping the
phase-end -> AG handoff ~2us. A persistent PSUM bank takes DUMMY_N
keep-warm matmuls at each phase start so AllGather waits never idle the
PE past the HAM re-throttle window (read out via the dbg output so DCE
keeps them).
"""

import numpy as np
import ml_dtypes

import concourse.bass as bass
import concourse.tile as tile
from concourse import bacc, mybir
from concourse.bass_utils import run_bass_kernel_spmd

N_CORES = 8
B = 256          # batch
D0 = 1024        # input dim
D = 4096         # hidden dims (layers 1 and 2)
D3 = 10          # output dim
F = D // N_CORES # 512 features per core per hidden layer
KC0 = D0 // 128  # 8
KC = D // 128    # 32
MC = F // 128    # 4
N_ITERS = 24     # steps 2..25 (step 1 done in preamble)
FREE_ITERS = 19  # iterations with free-phase s3 update (steps 2..20)
DUMMY_N = 12     # keep-warm matmuls per phase

BF16 = mybir.dt.bfloat16
F32 = mybir.dt.float32
OP = mybir.AluOpType
RG = [list(range(N_CORES))]

_BUILD_CACHE: dict = {}


def _build(n_iters: int = N_ITERS, free_iters: int = FREE_ITERS,
           dummy_n: int = DUMMY_N):
    key = (n_iters, free_iters, dummy_n)
    if key in _BUILD_CACHE:
        return _BUILD_CACHE[key]

    nc = bacc.Bacc("TRN2", target_bir_lowering=False, debug=False,
                   num_devices=N_CORES, enable_asserts=False)

    # --- per-core external I/O (weights pre-arranged host-side) ---
    fw0c = nc.dram_tensor("fw0c", [128, KC0 * F], BF16, kind="ExternalInput")
    fw1c = nc.dram_tensor("fw1c", [128, KC * F], BF16, kind="ExternalInput")
    bw1c = nc.dram_tensor("bw1c", [128, KC * F], BF16, kind="ExternalInput")
    fw2r = nc.dram_tensor("fw2r", [128, KC * D3], BF16, kind="ExternalInput")
    bw2c = nc.dram_tensor("bw2c", [D3, F], BF16, kind="ExternalInput")
    rxT = nc.dram_tensor("rxT", [128, KC0 * B], BF16, kind="ExternalInput")
    yh = nc.dram_tensor("yh", [D3, B], F32, kind="ExternalInput")
    o1 = nc.dram_tensor("o1", [F, B], F32, kind="ExternalOutput")
    o2 = nc.dram_tensor("o2", [F, B], F32, kind="ExternalOutput")
    o3 = nc.dram_tensor("o3", [D3, B], F32, kind="ExternalOutput")
    dbg = nc.dram_tensor("dbg", [128, 8], F32, kind="ExternalOutput")

    with tile.TileContext(nc) as tc:
        with tc.tile_pool(name="wp", bufs=1) as wp, \
             tc.tile_pool(name="st", bufs=1) as st, \
             tc.tile_pool(name="wk", bufs=2) as wk, \
             tc.tile_pool(name="gp", bufs=2) as gp, \
             tc.tile_pool(name="pp", bufs=1, space="PSUM") as pp, \
             tc.tile_pool(name="dp", bufs=2, space="DRAM") as dp:

            # ---- persistent state ----
            s1 = st.tile([128, MC * B], BF16)
            s2 = st.tile([128, MC * B], BF16)
            cc_t = st.tile([128, MC * B], F32)   # C = 0.25*(rx@fw0) slice
            o1f = st.tile([128, MC * B], F32)
            o2f = st.tile([128, MC * B], F32)
            o3f = st.tile([D3, B], F32)
            warm = pp.tile([128, 512], F32, tag="warm", name="warm")
            warm_on = [False]

            def keepwarm(n):
                for _ in range(n):
                    nc.tensor.matmul(warm[:], w_fw1[:, 0:128], w_fw1[:, 0:512],
                                     start=not warm_on[0], stop=True,
                                     skip_group_check=True)
                    warm_on[0] = True

            # ---- stage 0: the zeros AG for s2 starts before anything ----
            nc.vector.memset(s2[:], 0.0)
            s3_cur = wk.tile([D3, B], BF16, tag="s3", name="s3")
            nc.vector.memset(s3_cur[:], 0.0)

            def ag_open(which):
                agin = dp.tile([F, B], BF16, tag=f"agin{which}",
                               name=f"agin{which}")
                return agin, agin.rearrange("(j p) b -> p j b", p=128)

            def stage_chunk(agin3, s_tile, m):
                nc.sync.dma_start(
                    agin3[:, m:m + 1, :],
                    s_tile[:, m * B:(m + 1) * B]
                    .rearrange("p (j b) -> p j b", b=B))

            def ag_close(which, agin):
                agout = dp.tile([D, B], BF16, tag=f"agout{which}",
                                name=f"agout{which}", addr_space="Shared")
                nc.gpsimd.collective_compute(
                    "AllGather", OP.bypass, replica_groups=RG,
                    ins=[agin.opt()], outs=[agout.opt()])
                g = gp.tile([128, KC * B], BF16, tag=f"g{which}",
                            name=f"g{which}")
                g3 = g[:].rearrange("p (j b) -> p j b", b=B)
                ago = agout.rearrange("(j p) b -> p j b", p=128)
                for q in range(8):
                    nc.sync.dma_start(g3[:, q * 4:(q + 1) * 4, :],
                                      ago[:, q * 4:(q + 1) * 4, :])
                return g

            agin2, agin23 = ag_open("2")
            for m in range(MC):
                stage_chunk(agin23, s2, m)
            g2_cur = ag_close("2", agin2)

            # ---- preamble: C and step-1 s1, then its AG ----
            w_fw0 = wp.tile([128, KC0 * F], BF16)
            nc.sync.dma_start(w_fw0[:], fw0c[:])
            t_rx = wp.tile([128, KC0 * B], BF16)
            nc.sync.dma_start(t_rx[:], rxT[:])
            agin1, agin13 = ag_open("1")
            for m in range(MC):
                psc = pp.tile([128, B], F32, tag=f"mm{m}", name="psc")
                for k in range(KC0):
                    nc.tensor.matmul(
                        psc[:],
                        w_fw0[:, k * F + m * 128: k * F + (m + 1) * 128],
                        t_rx[:, k * B:(k + 1) * B],
                        start=(k == 0), stop=(k == KC0 - 1))
                sm = slice(m * B, (m + 1) * B)
                nc.vector.tensor_scalar_mul(cc_t[:, sm], psc[:], 0.25)
                nc.vector.tensor_scalar(s1[:, sm], cc_t[:, sm], 0.0, 1.0,
                                        OP.max, OP.min)
                stage_chunk(agin13, s1, m)
            g1_cur = ag_close("1", agin1)

            # ---- big weights load (overlaps the first AllGathers) ----
            w_fw1 = wp.tile([128, KC * F], BF16)
            nc.sync.dma_start(w_fw1[:], fw1c[:])
            w_bw1 = wp.tile([128, KC * F], BF16)
            nc.sync.dma_start(w_bw1[:], bw1c[:])
            w_fw2 = wp.tile([128, KC * D3], BF16)
            nc.sync.dma_start(w_fw2[:], fw2r[:])
            w_bw2 = wp.tile([D3, F], BF16)
            nc.sync.dma_start(w_bw2[:], bw2c[:])
            t_yh = wp.tile([D3, B], F32)
            nc.sync.dma_start(t_yh[:], yh[:])

            def phase_a(g2, s3c, weak: bool, last: bool):
                """psP1 = s2g@bw1, ps3 = s2g@fw2; s1,s3 update; AG(s1)."""
                keepwarm(dummy_n)
                h1 = wk.tile([128, MC * B], F32, tag="h1", name="h1")
                nc.vector.scalar_tensor_tensor(h1[:], s1[:], 0.5, cc_t[:],
                                               OP.mult, OP.add)
                if not last:
                    agin, agin3 = ag_open("1")
                p3 = pp.tile([D3, B], F32, tag="p3", name="p3")
                s3n = None
                for m in range(MC):
                    pm = pp.tile([128, B], F32, tag=f"mm{m}", name=f"pa{m}")
                    for j in range(KC):
                        rhs = g2[:, j * B:(j + 1) * B]
                        nc.tensor.matmul(
                            pm[:],
                            w_bw1[:, j * F + m * 128: j * F + (m + 1) * 128],
                            rhs, start=(j == 0), stop=(j == KC - 1))
                        if m == 0:
                            nc.tensor.matmul(
                                p3[:], w_fw2[:, j * D3:(j + 1) * D3],
                                rhs, start=(j == 0), stop=(j == KC - 1))
                    sm = slice(m * B, (m + 1) * B)
                    u = wk.tile([128, B], F32, tag="u", name="u")
                    nc.vector.scalar_tensor_tensor(
                        u[:], pm[:], 0.25, h1[:, sm], OP.mult, OP.add)
                    dst = o1f if last else s1
                    nc.vector.tensor_scalar(dst[:, sm], u[:], 0.0, 1.0,
                                            OP.max, OP.min)
                    if not last:
                        stage_chunk(agin3, s1, m)
                    if m == 0:
                        # s3 update as soon as p3 is done
                        s3n = o3f if last else wk.tile([D3, B], BF16,
                                                       tag="s3", name="s3")
                        if weak:
                            u3 = wk.tile([D3, B], F32, tag="u3", name="u3")
                            nc.vector.scalar_tensor_tensor(
                                u3[:], p3[:], 0.5, t_yh[:], OP.mult, OP.add)
                            nc.vector.tensor_scalar(s3n[:], u3[:], 0.0, 1.0,
                                                    OP.max, OP.min)
                        else:
                            u3 = wk.tile([D3, B], F32, tag="u3", name="u3")
                            nc.vector.tensor_tensor(u3[:], p3[:], s3c[:],
                                                    OP.add)
                            v3 = wk.tile([D3, B], F32, tag="v3", name="v3")
                            nc.vector.tensor_scalar(v3[:], u3[:], 0.5, 0.0,
                                                    OP.mult, OP.max)
                            nc.vector.tensor_scalar_min(s3n[:], v3[:], 1.0)
                if last:
                    return None, s3n
                return ag_close("1", agin), s3n

            def phase_b(g1, s3c, last: bool):
                """psP2 = s1g@fw1 + s3@bw2; s2 update; AG(s2)."""
                keepwarm(dummy_n)
                h2 = wk.tile([128, MC * B], F32, tag="h2", name="h2")
                nc.vector.tensor_scalar_mul(h2[:], s2[:], 0.5)
                if not last:
                    agin, agin3 = ag_open("2")
                for m in range(MC):
                    pm = pp.tile([128, B], F32, tag=f"mm{m}", name=f"pb{m}")
                    for j in range(KC):
                        nc.tensor.matmul(
                            pm[:],
                            w_fw1[:, j * F + m * 128: j * F + (m + 1) * 128],
                            g1[:, j * B:(j + 1) * B],
                            start=(j == 0), stop=False)
                    nc.tensor.matmul(pm[:], w_bw2[:, m * 128:(m + 1) * 128],
                                     s3c[:], start=False, stop=True)
                    sm = slice(m * B, (m + 1) * B)
                    u = wk.tile([128, B], F32, tag="u", name="u")
                    nc.vector.scalar_tensor_tensor(
                        u[:], pm[:], 0.25, h2[:, sm], OP.mult, OP.add)
                    dst = o2f if last else s2
                    nc.vector.tensor_scalar(dst[:, sm], u[:], 0.0, 1.0,
                                            OP.max, OP.min)
                    if not last:
                        stage_chunk(agin3, s2, m)
                if last:
                    return None
                return ag_close("2", agin)

            for t in range(n_iters):
                weak = t >= free_iters
                last = t == n_iters - 1
                if t % 2 == 0:
                    g1_next, s3_next = phase_a(g2_cur, s3_cur, weak, last)
                    g2_next = phase_b(g1_cur, s3_cur, last)
                else:
                    g2_next = phase_b(g1_cur, s3_cur, last)
                    g1_next, s3_next = phase_a(g2_cur, s3_cur, weak, last)
                g1_cur, g2_cur, s3_cur = g1_next, g2_next, s3_next

            # ---- outputs ----
            nc.sync.dma_start(o1.ap().rearrange("(j p) b -> p j b", p=128),
                              o1f[:].rearrange("p (j b) -> p j b", b=B))
            nc.sync.dma_start(o2.ap().rearrange("(j p) b -> p j b", p=128),
                              o2f[:].rearrange("p (j b) -> p j b", b=B))
            nc.sync.dma_start(o3.ap(), o3f[:])
            dbg_sb = st.tile([128, 8], F32)
            if dummy_n > 0:
                nc.vector.tensor_copy(dbg_sb[:], warm[:, 0:8])
            else:
                nc.vector.memset(dbg_sb[:], 0.0)
            nc.sync.dma_start(dbg.ap(), dbg_sb[:])

    nc.compile()
    _BUILD_CACHE[key] = nc
    return nc


def _rearr_w(w: np.ndarray, kc: int) -> np.ndarray:
    """[kc*128, M] -> [128, kc*M] with chunk k at cols [k*M,(k+1)*M)."""
    n, m = w.shape
    assert n == kc * 128
    return np.ascontiguousarray(
        w.reshape(kc, 128, m).transpose(1, 0, 2).reshape(128, kc * m))


def _prep_in_maps(x, fw0, fw1, fw2, bw1, bw2, y_one_hot):
    bf = ml_dtypes.bfloat16
    x = np.asarray(x, np.float32)
    rxT = np.clip(x, 0.0, 1.0).T.astype(np.float32)        # [1024, 256]
    rxT_r = _rearr_w(rxT, KC0).astype(bf)                   # [128, 8*256]
    fw2_r = _rearr_w(np.asarray(fw2, np.float32), KC).astype(bf)
    yh = (0.5 * np.asarray(y_one_hot, np.float32).T).astype(np.float32)
    yh = np.ascontiguousarray(yh)
    in_maps = []
    for c in range(N_CORES):
        sl = slice(c * F, (c + 1) * F)
        in_maps.append({
            "fw0c": _rearr_w(np.asarray(fw0, np.float32)[:, sl], KC0).astype(bf),
            "fw1c": _rearr_w(np.asarray(fw1, np.float32)[:, sl], KC).astype(bf),
            "bw1c": _rearr_w(np.asarray(bw1, np.float32)[:, sl], KC).astype(bf),
            "fw2r": fw2_r,
            "bw2c": np.ascontiguousarray(np.asarray(bw2, np.float32)[:, sl]).astype(bf),
            "rxT": rxT_r,
            "yh": yh,
        })
    return in_maps


def _assemble(results) -> np.ndarray:
    s1 = np.concatenate([results[c]["o1"] for c in range(N_CORES)], axis=0).T
    s2 = np.concatenate([results[c]["o2"] for c in range(N_CORES)], axis=0).T
    s3 = results[0]["o3"].T
    return np.ascontiguousarray(
        np.concatenate([s1, s2, s3], axis=1).astype(np.float32))


def run(inputs: dict, trace: bool = False, n_iters: int = N_ITERS,
        free_iters: int = FREE_ITERS, dummy_n: int = DUMMY_N):
    """Returns (output [256, 8202] fp32, BassKernelResults)."""
    nc = _build(n_iters, free_iters, dummy_n)
    in_maps = _prep_in_maps(
        inputs["x"], inputs["fw0"], inputs["fw1"], inputs["fw2"],
        inputs["bw1"], inputs["bw2"], inputs["y_one_hot"])
    r = run_bass_kernel_spmd(nc, in_maps, core_ids=list(range(N_CORES)),
                             trace=trace)
    return _assemble(r.results), r


def kernel(**inputs) -> np.ndarray:
    out, _ = run(inputs)
    return out


# revision 8
# speedup vs baseline: 1.1083x; 1.1083x over previous
"""Trainium2 Bass kernel for nn_BidirectionalMLP (8-core SPMD).

Math (from the reference, EPS=0.5, BETA=0.5):
  states stay in [0,1] after every clipped update, so rho(s)=s for all
  state tensors; rx = clip(x,0,1) is fixed.
  Per relaxation step:
    s1' = clip(0.5*s1 + 0.25*(rx@fw0) + 0.25*(s2@bw1), 0, 1)
    s2' = clip(0.5*s2 + 0.25*(s1@fw1 + s3@bw2), 0, 1)
    s3' = clip(0.5*s3 + 0.5*(s2@fw2), 0, 1)              (free phase)
    s3' = clip(0.5*(s2@fw2) + 0.5*y, 0, 1)               (weak phase)
  20 free steps + 5 weak steps from zero states. Step 1 is degenerate:
  s1(1) = clip(0.25*(rx@fw0)), s2(1) = 0, s3(1) = 0 — computed in the
  preamble, so the main loop runs 24 iterations.

Sharding: output-feature sharding of the big weights. Core c owns columns
[512c, 512c+512) of fw1/bw1/bw2, keeping fw1+bw1 SBUF-resident as bf16
(8MB). Each step all-gathers the bf16 states s1 and s2 (two AllGathers
of [512,256] per rank). Phase order alternates each iteration so each
AllGather has a full matmul phase to hide behind:
  phase A: psP1 = s2g@bw1 (+ ps3 = s2g@fw2) -> s1,s3 update -> AG(s1)
  phase B: psP2 = s1g@fw1 + s3@bw2          -> s2 update    -> AG(s2)
Matmuls are weight-stationary: out[feat,batch] tiles [128,256] fp32 in
PSUM, lhsT = weight chunk [128,128] bf16, rhs = gathered state chunk
[128,256] bf16. Loops run m-outer/j-inner so each 128-feature group
finishes early; its 2-op DVE epilogue writes the bf16 state in place and
immediately DMA-stages that chunk for the AllGather, keeping the
phase-end -> AG handoff ~2us. A persistent PSUM bank takes DUMMY_N
keep-warm matmuls at each phase start so AllGather waits never idle the
PE past the HAM re-throttle window (read out via the dbg output so DCE
keeps them).
"""

import numpy as np
import ml_dtypes

import concourse.bass as bass
import concourse.tile as tile
from concourse import bacc, mybir
from concourse.bass_utils import run_bass_kernel_spmd

N_CORES = 8
B = 256          # batch
D0 = 1024        # input dim
D = 4096         # hidden dims (layers 1 and 2)
D3 = 10          # output dim
F = D // N_CORES # 512 features per core per hidden layer
KC0 = D0 // 128  # 8
KC = D // 128    # 32
MC = F // 128    # 4
N_ITERS = 24     # steps 2..25 (step 1 done in preamble)
FREE_ITERS = 19  # iterations with free-phase s3 update (steps 2..20)
DUMMY_N = 12     # keep-warm matmuls per phase

BF16 = mybir.dt.bfloat16
FP8 = mybir.dt.float8e4
F32 = mybir.dt.float32
OP = mybir.AluOpType
RG = [list(range(N_CORES))]

_BUILD_CACHE: dict = {}


def _build(n_iters: int = N_ITERS, free_iters: int = FREE_ITERS,
           dummy_n: int = DUMMY_N):
    key = (n_iters, free_iters, dummy_n)
    if key in _BUILD_CACHE:
        return _BUILD_CACHE[key]

    nc = bacc.Bacc("TRN2", target_bir_lowering=False, debug=False,
                   num_devices=N_CORES, enable_asserts=False)

    # --- per-core external I/O (weights pre-arranged host-side) ---
    fw0c = nc.dram_tensor("fw0c", [128, KC0 * F], BF16, kind="ExternalInput")
    fw1c = nc.dram_tensor("fw1c", [128, KC * F], BF16, kind="ExternalInput")
    bw1c = nc.dram_tensor("bw1c", [128, KC * F], BF16, kind="ExternalInput")
    fw2r = nc.dram_tensor("fw2r", [128, KC * D3], BF16, kind="ExternalInput")
    bw2c = nc.dram_tensor("bw2c", [D3, F], BF16, kind="ExternalInput")
    rxT = nc.dram_tensor("rxT", [128, KC0 * B], BF16, kind="ExternalInput")
    yh = nc.dram_tensor("yh", [D3, B], F32, kind="ExternalInput")
    o1 = nc.dram_tensor("o1", [F, B], F32, kind="ExternalOutput")
    o2 = nc.dram_tensor("o2", [F, B], F32, kind="ExternalOutput")
    o3 = nc.dram_tensor("o3", [D3, B], F32, kind="ExternalOutput")
    dbg = nc.dram_tensor("dbg", [128, 8], F32, kind="ExternalOutput")

    with tile.TileContext(nc) as tc:
        with tc.tile_pool(name="wp", bufs=1) as wp, \
             tc.tile_pool(name="st", bufs=1) as st, \
             tc.tile_pool(name="wk", bufs=2) as wk, \
             tc.tile_pool(name="gp", bufs=2) as gp, \
             tc.tile_pool(name="pp", bufs=1, space="PSUM") as pp, \
             tc.tile_pool(name="dp", bufs=2, space="DRAM") as dp:

            # ---- persistent state ----
            s1 = st.tile([128, MC * B], BF16)
            s2 = st.tile([128, MC * B], BF16)
            cc_t = st.tile([128, MC * B], F32)   # C = 0.25*(rx@fw0) slice
            o1f = st.tile([128, MC * B], F32)
            o2f = st.tile([128, MC * B], F32)
            o3f = st.tile([D3, B], F32)
            warm = pp.tile([128, 512], F32, tag="warm", name="warm")
            warm_on = [False]

            def keepwarm(n):
                for _ in range(n):
                    nc.tensor.matmul(warm[:], w_fw1[:, 0:128], w_fw1[:, 0:512],
                                     start=not warm_on[0], stop=True,
                                     skip_group_check=True)
                    warm_on[0] = True

            # ---- stage 0: the zeros AG for s2 starts before anything ----
            nc.vector.memset(s2[:], 0.0)
            s3_cur = wk.tile([D3, B], BF16, tag="s3", name="s3")
            nc.vector.memset(s3_cur[:], 0.0)

            def ag_open(which):
                agin = dp.tile([F, B], FP8, tag=f"agin{which}",
                               name=f"agin{which}")
                return agin, agin.rearrange("(j p) b -> p j b", p=128)

            def stage_chunk(agin3, q_tile, m):
                nc.sync.dma_start(
                    agin3[:, m:m + 1, :],
                    q_tile[:, m * B:(m + 1) * B]
                    .rearrange("p (j b) -> p j b", b=B))

            def ag_close(which, agin):
                agout = dp.tile([D, B], FP8, tag=f"agout{which}",
                                name=f"agout{which}", addr_space="Shared")
                nc.gpsimd.collective_compute(
                    "AllGather", OP.bypass, replica_groups=RG,
                    ins=[agin.opt()], outs=[agout.opt()])
                g = gp.tile([128, KC * B], FP8, tag=f"g{which}",
                            name=f"g{which}")
                g3 = g[:].rearrange("p (j b) -> p j b", b=B)
                ago = agout.rearrange("(j p) b -> p j b", p=128)
                for q in range(8):
                    nc.sync.dma_start(g3[:, q * 4:(q + 1) * 4, :],
                                      ago[:, q * 4:(q + 1) * 4, :])
                return g

            sq2_z = wk.tile([128, MC * B], FP8, tag="sq2", name="sq2")
            nc.vector.memset(sq2_z[:], 0.0)
            agin2, agin23 = ag_open("2")
            for m in range(MC):
                stage_chunk(agin23, sq2_z, m)
            g2_cur = ag_close("2", agin2)

            # ---- preamble: C and step-1 s1, then its AG ----
            w_fw0 = wp.tile([128, KC0 * F], BF16)
            nc.sync.dma_start(w_fw0[:], fw0c[:])
            t_rx = wp.tile([128, KC0 * B], BF16)
            nc.sync.dma_start(t_rx[:], rxT[:])
            agin1, agin13 = ag_open("1")
            sq1_p = wk.tile([128, MC * B], FP8, tag="sq1", name="sq1")
            for m in range(MC):
                psc = pp.tile([128, B], F32, tag=f"mm{m}", name="psc")
                for k in range(KC0):
                    nc.tensor.matmul(
                        psc[:],
                        w_fw0[:, k * F + m * 128: k * F + (m + 1) * 128],
                        t_rx[:, k * B:(k + 1) * B],
                        start=(k == 0), stop=(k == KC0 - 1))
                sm = slice(m * B, (m + 1) * B)
                nc.vector.tensor_scalar_mul(cc_t[:, sm], psc[:], 0.25)
                nc.vector.tensor_scalar(s1[:, sm], cc_t[:, sm], 0.0, 1.0,
                                        OP.max, OP.min)
                nc.scalar.copy(sq1_p[:, sm], s1[:, sm])
                stage_chunk(agin13, sq1_p, m)
            g1_cur = ag_close("1", agin1)

            # ---- big weights load (overlaps the first AllGathers) ----
            w_fw1 = wp.tile([128, KC * F], BF16)
            nc.sync.dma_start(w_fw1[:], fw1c[:])
            w_bw1 = wp.tile([128, KC * F], BF16)
            nc.sync.dma_start(w_bw1[:], bw1c[:])
            w_fw2 = wp.tile([128, KC * D3], BF16)
            nc.sync.dma_start(w_fw2[:], fw2r[:])
            w_bw2 = wp.tile([D3, F], BF16)
            nc.sync.dma_start(w_bw2[:], bw2c[:])
            t_yh = wp.tile([D3, B], F32)
            nc.sync.dma_start(t_yh[:], yh[:])

            def phase_a(g2, s3c, weak: bool, last: bool):
                """psP1 = s2g@bw1, ps3 = s2g@fw2; s1,s3 update; AG(s1)."""
                keepwarm(dummy_n)
                h1 = wk.tile([128, MC * B], F32, tag="h1", name="h1")
                nc.vector.scalar_tensor_tensor(h1[:], s1[:], 0.5, cc_t[:],
                                               OP.mult, OP.add)
                if not last:
                    agin, agin3 = ag_open("1")
                    sq1 = wk.tile([128, MC * B], FP8, tag="sq1", name="sq1")
                p3 = pp.tile([D3, B], F32, tag="p3", name="p3")
                s3n = None
                for m in range(MC):
                    pm = pp.tile([128, B], F32, tag=f"mm{m}", name=f"pa{m}")
                    for j in range(KC):
                        rhs = g2[:, j * B:(j + 1) * B]
                        nc.tensor.matmul(
                            pm[:],
                            w_bw1[:, j * F + m * 128: j * F + (m + 1) * 128],
                            rhs, start=(j == 0), stop=(j == KC - 1))
                        if m == 0:
                            nc.tensor.matmul(
                                p3[:], w_fw2[:, j * D3:(j + 1) * D3],
                                rhs, start=(j == 0), stop=(j == KC - 1))
                    sm = slice(m * B, (m + 1) * B)
                    u = wk.tile([128, B], F32, tag="u", name="u")
                    nc.vector.scalar_tensor_tensor(
                        u[:], pm[:], 0.25, h1[:, sm], OP.mult, OP.add)
                    dst = o1f if last else s1
                    nc.vector.tensor_scalar(dst[:, sm], u[:], 0.0, 1.0,
                                            OP.max, OP.min)
                    if not last:
                        nc.scalar.copy(sq1[:, sm], s1[:, sm])
                        stage_chunk(agin3, sq1, m)
                    if m == 0:
                        # s3 update as soon as p3 is done
                        s3n = o3f if last else wk.tile([D3, B], BF16,
                                                       tag="s3", name="s3")
                        if weak:
                            u3 = wk.tile([D3, B], F32, tag="u3", name="u3")
                            nc.vector.scalar_tensor_tensor(
                                u3[:], p3[:], 0.5, t_yh[:], OP.mult, OP.add)
                            nc.vector.tensor_scalar(s3n[:], u3[:], 0.0, 1.0,
                                                    OP.max, OP.min)
                        else:
                            u3 = wk.tile([D3, B], F32, tag="u3", name="u3")
                            nc.vector.tensor_tensor(u3[:], p3[:], s3c[:],
                                                    OP.add)
                            v3 = wk.tile([D3, B], F32, tag="v3", name="v3")
                            nc.vector.tensor_scalar(v3[:], u3[:], 0.5, 0.0,
                                                    OP.mult, OP.max)
                            nc.vector.tensor_scalar_min(s3n[:], v3[:], 1.0)
                if last:
                    return None, s3n
                return ag_close("1", agin), s3n

            def phase_b(g1, s3c, last: bool):
                """psP2 = s1g@fw1 + s3@bw2; s2 update; AG(s2)."""
                keepwarm(dummy_n)
                h2 = wk.tile([128, MC * B], F32, tag="h2", name="h2")
                nc.vector.tensor_scalar_mul(h2[:], s2[:], 0.5)
                if not last:
                    agin, agin3 = ag_open("2")
                    sq2 = wk.tile([128, MC * B], FP8, tag="sq2", name="sq2")
                for m in range(MC):
                    pm = pp.tile([128, B], F32, tag=f"mm{m}", name=f"pb{m}")
                    for j in range(KC):
                        nc.tensor.matmul(
                            pm[:],
                            w_fw1[:, j * F + m * 128: j * F + (m + 1) * 128],
                            g1[:, j * B:(j + 1) * B],
                            start=(j == 0), stop=False)
                    nc.tensor.matmul(pm[:], w_bw2[:, m * 128:(m + 1) * 128],
                                     s3c[:], start=False, stop=True)
                    sm = slice(m * B, (m + 1) * B)
                    u = wk.tile([128, B], F32, tag="u", name="u")
                    nc.vector.scalar_tensor_tensor(
                        u[:], pm[:], 0.25, h2[:, sm], OP.mult, OP.add)
                    dst = o2f if last else s2
                    nc.vector.tensor_scalar(dst[:, sm], u[:], 0.0, 1.0,
                                            OP.max, OP.min)
                    if not last:
                        nc.scalar.copy(sq2[:, sm], s2[:, sm])
                        stage_chunk(agin3, sq2, m)
                if last:
                    return None
                return ag_close("2", agin)

            for t in range(n_iters):
                weak = t >= free_iters
                last = t == n_iters - 1
                if t % 2 == 0:
                    g1_next, s3_next = phase_a(g2_cur, s3_cur, weak, last)
                    g2_next = phase_b(g1_cur, s3_cur, last)
                else:
                    g2_next = phase_b(g1_cur, s3_cur, last)
                    g1_next, s3_next = phase_a(g2_cur, s3_cur, weak, last)
                g1_cur, g2_cur, s3_cur = g1_next, g2_next, s3_next

            # ---- outputs ----
            nc.sync.dma_start(o1.ap().rearrange("(j p) b -> p j b", p=128),
                              o1f[:].rearrange("p (j b) -> p j b", b=B))
            nc.sync.dma_start(o2.ap().rearrange("(j p) b -> p j b", p=128),
                              o2f[:].rearrange("p (j b) -> p j b", b=B))
            nc.sync.dma_start(o3.ap(), o3f[:])
            dbg_sb = st.tile([128, 8], F32)
            if dummy_n > 0:
                nc.vector.tensor_copy(dbg_sb[:], warm[:, 0:8])
            else:
                nc.vector.memset(dbg_sb[:], 0.0)
            nc.sync.dma_start(dbg.ap(), dbg_sb[:])

    nc.compile()
    _BUILD_CACHE[key] = nc
    return nc


def _rearr_w(w: np.ndarray, kc: int) -> np.ndarray:
    """[kc*128, M] -> [128, kc*M] with chunk k at cols [k*M,(k+1)*M)."""
    n, m = w.shape
    assert n == kc * 128
    return np.ascontiguousarray(
        w.reshape(kc, 128, m).transpose(1, 0, 2).reshape(128, kc * m))


def _prep_in_maps(x, fw0, fw1, fw2, bw1, bw2, y_one_hot):
    bf = ml_dtypes.bfloat16
    x = np.asarray(x, np.float32)
    rxT = np.clip(x, 0.0, 1.0).T.astype(np.float32)        # [1024, 256]
    rxT_r = _rearr_w(rxT, KC0).astype(bf)                   # [128, 8*256]
    fw2_r = _rearr_w(np.asarray(fw2, np.float32), KC).astype(bf)
    yh = (0.5 * np.asarray(y_one_hot, np.float32).T).astype(np.float32)
    yh = np.ascontiguousarray(yh)
    in_maps = []
    for c in range(N_CORES):
        sl = slice(c * F, (c + 1) * F)
        in_maps.append({
            "fw0c": _rearr_w(np.asarray(fw0, np.float32)[:, sl], KC0).astype(bf),
            "fw1c": _rearr_w(np.asarray(fw1, np.float32)[:, sl], KC).astype(bf),
            "bw1c": _rearr_w(np.asarray(bw1, np.float32)[:, sl], KC).astype(bf),
            "fw2r": fw2_r,
            "bw2c": np.ascontiguousarray(np.asarray(bw2, np.float32)[:, sl]).astype(bf),
            "rxT": rxT_r,
            "yh": yh,
        })
    return in_maps


def _assemble(results) -> np.ndarray:
    s1 = np.concatenate([results[c]["o1"] for c in range(N_CORES)], axis=0).T
    s2 = np.concatenate([results[c]["o2"] for c in range(N_CORES)], axis=0).T
    s3 = results[0]["o3"].T
    return np.ascontiguousarray(
        np.concatenate([s1, s2, s3], axis=1).astype(np.float32))


def run(inputs: dict, trace: bool = False, n_iters: int = N_ITERS,
        free_iters: int = FREE_ITERS, dummy_n: int = DUMMY_N):
    """Returns (output [256, 8202] fp32, BassKernelResults)."""
    nc = _build(n_iters, free_iters, dummy_n)
    in_maps = _prep_in_maps(
        inputs["x"], inputs["fw0"], inputs["fw1"], inputs["fw2"],
        inputs["bw1"], inputs["bw2"], inputs["y_one_hot"])
    r = run_bass_kernel_spmd(nc, in_maps, core_ids=list(range(N_CORES)),
                             trace=trace)
    return _assemble(r.results), r


def kernel(**inputs) -> np.ndarray:
    out, _ = run(inputs)
    return out
